# revision 18
# baseline (speedup 1.0000x reference)
"""Multi-head attention (B=2, S=2048, D=1024, H=16, Dk=64) on 8 NeuronCores.

Sharding: 2-way data parallel over batch x 4-way tensor parallel over heads.
Core c = 4*b + g handles batch b, head group g (4 heads = 256 cols).
W_o is row-sliced; host sums the 4 partial outputs per batch (+ bv@Wo + bo).

v3 (elementwise-bound rework; ACT/DVE are the binding engines):
  - Q/K projections: fp8 DoubleRow as before, but the W-column permutation is
    chosen so each m-tile's psum [128,512] maps partition-straight into a
    single [128,2,S] qt8/kt8 tile (head h at partition base 32h, legal
    quadrant bases). The psum->sbuf move is ONE full-partition op per
    projection: DVE tensor_tensor add with a broadcast per-(p,m) bias tile;
    no /16 rescale (scales folded into the exp constant).
  - scores: per-head DR with dk=64 at quadrant base 32h (unchanged math).
  - softmax exp: ACT exact Exp and DVE Schraudolph split by weighted
    round-robin, scale = 0.125/256.
  - P@V FLIPPED: stationary = probs chunk [128,2,128], moving = vaug
    [128,2,65] (64 V-dims + 1 rowsum const col per head); out psum
    [128 tok, 4, 65]. Full A+R residual compensation (V stored as fp8 A
    plus equal-scale fp8 residual R). 65-wide outputs cut P@V PE time by
    ~35% and shrink the normalization to [128,4,64] broadcast ops.
  - normalization: DVE reciprocal [128,4] + one broadcast tensor_tensor mul
    writing bf16 attnP; DMA-transpose ([128,128] bf16 tiles, 14ns/xbar-tile)
    rebuilds the [dims, tok] layout for the output projection.
  - V path: V-proj psum -> ONE bf16 copy per qtr; Pool (gpsimd) builds fp8
    A and R from SBUF via tensor_tensor (GPSIMD cannot touch PSUM).
  - output projection: flipped (out^T [D,S] in DRAM, host transposes),
    moving = ot_t [128,2,512] bf16, stationary = Wo; psum->sbuf copies on
    ACT. bq/bk biases on-chip; bv/bo folded into the host-side reduction.
"""
import numpy as np
import ml_dtypes
from contextlib import ExitStack

import concourse.bass as bass
import concourse.mybir as mybir
import concourse.tile as tile
from concourse import bacc
from concourse.bass_utils import run_bass_kernel_spmd

F32 = mybir.dt.float32
F8 = mybir.dt.float8e4
I8 = mybir.dt.int8
BF = mybir.dt.bfloat16
E4 = ml_dtypes.float8_e4m3
BF16 = ml_dtypes.bfloat16
DR = mybir.MatmulPerfMode.DoubleRow

B, S, D = 2, 2048, 1024
P = 128
W = 256              # local width (4 heads x 64)
MT = 2               # m-tiles of 128
KO8 = D // P         # 8 bf16 contraction ktiles
KO4 = D // 256       # 4 DoubleRow ktiles
NQ = S // 512        # 4 sq chunks
NPAIR = S // 256     # 8 sk chunk-pairs
HW = 65              # per-head moving width in P@V (64 dims + rowsum col)
SCALE = 0.125 / 256.0          # qt8/kt8 carry x16 each
A_SCH = 8.0 / np.log(2.0) * SCALE
B_SCH = 55.6

_CACHE = {}


# exp engine split: deterministic weighted round-robin (Bresenham).
class _ExpAssign:
    def __init__(self, wa, wd):
        self.w = {"A": wa, "D": wd}
        self.acc = {"A": 0.0, "D": 0.0}

    def next(self):
        for k in self.acc:
            self.acc[k] += self.w[k]
        k = max(self.acc, key=lambda e: self.acc[e])
        self.acc[k] -= sum(self.w.values())
        return k


def build_nc():
    nc = bacc.Bacc("TRN2", target_bir_lowering=False, debug=False, num_devices=8)
    xb = nc.dram_tensor("xb", [D, S], BF, kind="ExternalInput").ap()
    x8 = nc.dram_tensor("x8", [D, S], F8, kind="ExternalInput").ap()
    wq8 = nc.dram_tensor("wq8", [D, W], F8, kind="ExternalInput").ap()
    wk8 = nc.dram_tensor("wk8", [D, W], F8, kind="ExternalInput").ap()
    wv = nc.dram_tensor("wv", [D, W], BF, kind="ExternalInput").ap()
    wo = nc.dram_tensor("wo", [W, D], BF, kind="ExternalInput").ap()
    bq = nc.dram_tensor("bq", [P, MT], F32, kind="ExternalInput").ap()
    bk = nc.dram_tensor("bk", [P, MT], F32, kind="ExternalInput").ap()
    out = nc.dram_tensor("out", [D, S], BF, kind="ExternalOutput").ap()

    x8_r = x8.rearrange("(ko t p) s -> p ko t s", p=P, t=2)
    xb_r = xb.rearrange("(ko p) s -> p ko s", p=P)
    wq8_r = wq8.rearrange("(ko t p) w -> p ko t w", p=P, t=2)
    wk8_r = wk8.rearrange("(ko t p) w -> p ko t w", p=P, t=2)
    wv_r = wv.rearrange("(ko p) w -> p ko w", p=P)
    wo_r = wo.rearrange("(m p) d -> p m d", p=P)

    with tile.TileContext(nc) as tc, ExitStack() as ctx:
        sb = ctx.enter_context(tc.tile_pool(name="sb", bufs=1))
        xp = ctx.enter_context(tc.tile_pool(name="xp", bufs=2))
        ptp = ctx.enter_context(tc.tile_pool(name="ptp", bufs=4))
        cp = ctx.enter_context(tc.tile_pool(name="cp", bufs=8))
        ps = ctx.enter_context(tc.tile_pool(name="ps", bufs=1, space="PSUM"))

        # ---- resident inputs (issue order = DMA priority) ----
        w8q_t = sb.tile([P, KO4, 2, W], F8)
        nc.scalar.dma_start(w8q_t[:], wq8_r)
        x8_tiles = [xp.tile([P, KO4, 2, 512], F8, tag="x8", name=f"x8_{i}",
                            bufs=4) for i in range(4)]

        def load_x8(qtr):
            sq = slice(qtr * 512, (qtr + 1) * 512)
            nc.sync.dma_start(x8_tiles[qtr][:, 0:2, :, :], x8_r[:, 0:2, :, sq])
            nc.sync.dma_start(x8_tiles[qtr][:, 2:4, :, :], x8_r[:, 2:4, :, sq])

        sq0 = slice(0, 512)
        nc.sync.dma_start(x8_tiles[0][:, 0:2, :, :], x8_r[:, 0:2, :, sq0])
        bq_t = sb.tile([P, MT], F32)
        nc.sync.dma_start(bq_t[:], bq)
        nc.sync.dma_start(x8_tiles[0][:, 2:4, :, :], x8_r[:, 2:4, :, sq0])
        w8k_t = sb.tile([P, KO4, 2, W], F8)
        nc.scalar.dma_start(w8k_t[:], wk8_r)
        bk_t = sb.tile([P, MT], F32)
        nc.sync.dma_start(bk_t[:], bk)

        xq_tiles = [None] * 4

        def load_xq(qtr):
            sq = slice(qtr * 512, (qtr + 1) * 512)
            xq_tiles[qtr] = xp.tile([P, KO8, 512], BF, tag="xq", bufs=3,
                                    name=f"xq{qtr}")
            nc.sync.dma_start(xq_tiles[qtr][:, 0:KO8 // 2, :],
                              xb_r[:, 0:KO8 // 2, sq])
            nc.sync.dma_start(xq_tiles[qtr][:, KO8 // 2:KO8, :],
                              xb_r[:, KO8 // 2:KO8, sq])

        load_xq(0)
        wv_t = sb.tile([P, KO8, W], BF)
        nc.sync.dma_start(wv_t[:], wv_r)
        for _q in (1, 2, 3):
            load_x8(_q)
        load_xq(1)
        wo_t = sb.tile([P, MT, D], BF)
        nc.sync.dma_start(wo_t[:], wo_r)

        zt = sb.tile([P, 1], BF)
        nc.vector.memset(zt[:], 0.0)

        # qt8/kt8: two [64, m, S] tiles (head-pairs); within a tile head
        # h%2 sits at base 32*(h%2) (base 96 is illegal for matmul operands),
        # dk = 32m + p%32
        qt8_m = [sb.tile([64, MT, S], F8, name=f"qt8_{j}") for j in range(2)]
        kt8_m = [sb.tile([64, MT, S], F8, name=f"kt8_{j}") for j in range(2)]
        # vaug A/R: [sk-part, pair, parity, head*65]; per head 64 V-dims
        # then 1 rowsum col (A: 16.0; R: 0.0)
        vaugA_t = sb.tile([P, NPAIR, 2, 4 * HW], F8)
        vaugR_t = sb.tile([P, NPAIR, 2, 4 * HW], F8)
        consA = vaugA_t[:].rearrange("p i t (h c) -> p i t h c", c=HW)[:, :, :, :, 64:HW]
        nc.gpsimd.memset(consA, 16.0)
        consR = vaugR_t[:].rearrange("p i t (h c) -> p i t h c", c=HW)[:, :, :, :, 64:HW]
        nc.gpsimd.memset(consR, 0.0)
        ot_t = sb.tile([P, MT, S], BF)

        ea_p1 = _ExpAssign(1.0, 1.0)      # phase-1 blocks
        ea_p2 = _ExpAssign(1.05, 1.0)     # phase-2 mix

        _uid = [0]
        pending_out = []
        pending_tp = []
        pending_dma = []

        # attnP per q-block: [tok-part, chunk, head-pair, 128 dims] bf16
        def attn_block(q, h, ea, attnP, defer=False):
            _uid[0] += 1
            u = _uid[0]
            sq = slice(q * 512, (q + 1) * 512)
            hq = slice(32 * (h % 2), 32 * (h % 2) + 32)
            qt8, kt8 = qt8_m[h // 2], kt8_m[h // 2]
            hc = slice(h * HW, (h + 1) * HW)
            oPbox = [None]

            def attn_v(i, pt):
                if oPbox[0] is None:
                    oPt = ps.tile([P, 512], F32, tag="oP", bufs=2,
                                  name=f"oP{u}")
                    oPbox[0] = oPt[:].rearrange(
                        "p (c n) -> p c n", n=128)[:, :, 0:HW]
                oP = oPbox[0]
                for c in range(4):
                    stat = pt[:, :, c * 128:(c + 1) * 128]
                    nc.tensor.matmul(oP[:, c, :], stat, vaugA_t[:, i, :, hc],
                                     start=(i == 0), stop=False,
                                     perf_mode=DR)
                    nc.tensor.matmul(oP[:, c, :], stat, vaugR_t[:, i, :, hc],
                                     start=False, stop=(i == NPAIR - 1),
                                     perf_mode=DR)

            pend_av = []
            for i in range(NPAIR):
                pt = ptp.tile([P, 2, 512], F8, tag="pt", bufs=28,
                              name=f"pt{u}_{i}")
                sp = ps.tile([P, 1024], F32, tag="sp", bufs=3,
                             name=f"sp{u}_{i}")
                for half in (0, 1):
                    ks = slice((2 * i + half) * P, (2 * i + half + 1) * P)
                    nc.tensor.matmul(sp[:, half * 512:(half + 1) * 512],
                                     kt8[hq, :, ks], qt8[hq, :, sq],
                                     start=True, stop=True, perf_mode=DR)
                ptf = pt[:].rearrange("p t n -> p (t n)")
                eng = ea.next()
                if eng == "A":
                    nc.scalar.activation(ptf, sp[:],
                                         mybir.ActivationFunctionType.Exp,
                                         bias=0.0, scale=SCALE)
                else:
                    nc.vector.tensor_scalar(ptf.bitcast(I8), sp[:],
                                            A_SCH, B_SCH,
                                            mybir.AluOpType.mult,
                                            mybir.AluOpType.add)
                if not defer and len(pend_av) >= 2:
                    attn_v(*pend_av.pop(0))
                pend_av.append((i, pt))
                if pending_tp:
                    emit_tp(*pending_tp.pop(0))
                elif pending_dma:
                    emit_dma(*pending_dma.pop(0))
                if pending_out and i >= 4 and (i % 2 == 0
                                               or len(pending_out) >= 6):
                    outproj_unit(*pending_out.pop(0))
                yield i
            while pend_av:
                attn_v(*pend_av.pop(0))
            oP = oPbox[0]
            # normalization: rs = 1/rowsum; attnP slice = oP[:, :, 0:64]*rs
            rs = cp.tile([P, 4], F32, tag="rs", bufs=4, name=f"rs{u}")
            nc.vector.reciprocal(rs[:], oP[:, :, 64:HW].rearrange("p c j -> p (c j)"))
            nc.vector.tensor_tensor(
                attnP[:, :, h // 2, 64 * (h % 2):64 * (h % 2) + 64],
                oP[:, :, 0:64],
                rs[:].rearrange("p (c j) -> p c j", j=1).broadcast_to([P, 4, 64]),
                mybir.AluOpType.mult)
            yield NPAIR

        def emit_tp(q, hp, c, attnP):
            nc.sync.dma_start(
                ot_t[:, hp, q * 512 + c * 128: q * 512 + (c + 1) * 128],
                attnP[:, c, hp, :], transpose=True)
            if hp == 1 and c == 3:
                outproj(q)

        def transposes(q, hp, attnP):
            pending_tp.extend((q, hp, c, attnP) for c in range(4))

        def qk_proj(qtr, wt, bt, dst):
            sq = slice(qtr * 512, (qtr + 1) * 512)
            ppt = ps.tile([P, 1024], F32, tag="sp", bufs=3)
            for m in range(MT):
                for ko in range(KO4):
                    nc.tensor.matmul(ppt[:, m * 512:(m + 1) * 512],
                                     wt[:, ko, :, m * P:(m + 1) * P],
                                     x8_tiles[qtr][:, ko, :, :],
                                     start=(ko == 0), stop=(ko == KO4 - 1),
                                     perf_mode=DR)
            # head-pair 0 (psum parts 0:64) on DVE in one op; head-pair 1
            # (parts 64:128, partition-shifted) on ACT as two per-m ops
            nc.vector.tensor_tensor(
                dst[0][:, :, sq],
                ppt[0:64, :].rearrange("p (m n) -> p m n", n=512),
                bt[0:64, :].rearrange("p (m j) -> p m j", j=1)
                .broadcast_to([64, MT, 512]),
                mybir.AluOpType.add)
            for m in range(MT):
                nc.scalar.activation(
                    dst[1][:, m, sq], ppt[64:P, m * 512:(m + 1) * 512],
                    mybir.ActivationFunctionType.Identity,
                    bias=bt[64:P, m:m + 1], scale=1.0)

        def v_proj_mm(qtr, st2):
            # two st chunks of matmuls into the shared pvt tile
            if st2 == 0:
                pvt = ps.tile([P, 1024], F32, tag="sp", bufs=3)
                v_proj_mm.pvt = pvt
            else:
                pvt = v_proj_mm.pvt
            for st in (2 * st2, 2 * st2 + 1):
                pv = pvt[:, st * W:(st + 1) * W]
                for ko in range(KO8):
                    nc.tensor.matmul(pv, xq_tiles[qtr][:, ko, st * P:(st + 1) * P],
                                     wv_t[:, ko, :], start=(ko == 0),
                                     stop=(ko == KO8 - 1))
            return pvt

        def v_finish(qtr, pvt):
            vb = cp.tile([P, 1024], BF, tag="vb", bufs=2)
            nc.scalar.activation(vb[:], pvt[:],
                                 mybir.ActivationFunctionType.Copy,
                                 bias=0.0, scale=1.0)
            # Pool: A = fp8(vb), R = fp8(vb - A); layout [p, st(a b), h, 64]
            vbv = vb[:].rearrange("p (a b h c) -> p a b h c", a=2, b=2, c=64)
            Av = vaugA_t[:, 2 * qtr:2 * qtr + 2, :, :].rearrange(
                "p a b (h c) -> p a b h c", c=HW)[:, :, :, :, 0:64]
            Rv = vaugR_t[:, 2 * qtr:2 * qtr + 2, :, :].rearrange(
                "p a b (h c) -> p a b h c", c=HW)[:, :, :, :, 0:64]
            nc.gpsimd.tensor_tensor(Av, vbv,
                                    zt[:].rearrange("p (a b h c) -> p a b h c",
                                                    a=1, b=1, h=1)
                                    .broadcast_to([P, 2, 2, 4, 64]),
                                    mybir.AluOpType.add)
            nc.gpsimd.tensor_tensor(Rv, vbv, Av, mybir.AluOpType.subtract)

        _ob_cur = [None]

        def emit_dma(q, dc0, ob):
            sq = slice(q * 512, (q + 1) * 512)
            dr = out[dc0 * P:(dc0 + 2) * P, sq].rearrange(
                "(a p) n -> p a n", a=2)
            nc.sync.dma_start(dr, ob[:])

        def outproj_unit(q, dc, ceng):
            pot = ps.tile([P, 1024], F32, tag="sp", bufs=3, name="pot")
            po = pot[:, 0:512]
            sq = slice(q * 512, (q + 1) * 512)
            for m in range(MT):
                nc.tensor.matmul(po[:], wo_t[:, m, dc * P:(dc + 1) * P],
                                 ot_t[:, m, sq],
                                 start=(m == 0), stop=(m == MT - 1))
            if dc % 2 == 0:
                _ob_cur[0] = cp.tile([P, 2, 512], BF, tag="ob", bufs=3, name="ob")
            ob = _ob_cur[0]
            half = ob[:, dc % 2, :]
            if ceng == "A":
                nc.scalar.activation(half, po[:],
                                     mybir.ActivationFunctionType.Copy,
                                     bias=0.0, scale=1.0)
            else:
                nc.vector.tensor_copy(half, po[:])
            if dc % 2 == 1:
                pending_dma.append((q, dc - 1, ob))

        def outproj(q):
            for dc in range(8):
                pending_out.append((q, dc, "A" if dc % 4 != 3 else "D"))

        attnP_tiles = {}

        def get_attnP(q):
            if q not in attnP_tiles:
                attnP_tiles[q] = cp.tile([P, 4, 2, P], BF, tag="attnP",
                                         bufs=2, name=f"attnP{q}")
            return attnP_tiles[q]

        # ---- phase 1: projections with 4 interleaved blocks (2 with live
        # P@V accumulation, 2 scoring-only with deferred P@V) ----
        blk00 = attn_block(0, 0, ea_p1, get_attnP(0))
        blk01 = attn_block(0, 1, ea_p1, get_attnP(0))
        blk02 = attn_block(0, 2, ea_p1, get_attnP(0), defer=True)
        blk03 = attn_block(0, 3, ea_p1, get_attnP(0), defer=True)
        for qtr in range(NQ):
            if qtr > 1:
                load_xq(qtr)
            qk_proj(qtr, w8q_t, bq_t, qt8_m)
            next(blk00, None)
            next(blk02, None)
            qk_proj(qtr, w8k_t, bk_t, kt8_m)
            next(blk01, None)
            next(blk03, None)
            pvt = v_proj_mm(qtr, 0)
            next(blk00, None)
            next(blk02, None)
            v_proj_mm(qtr, 1)
            next(blk01, None)
            next(blk03, None)
            v_finish(qtr, pvt)
        next(blk00, None)   # epilogues: active blocks first
        next(blk01, None)
        for _ in blk00:
            pass
        for _ in blk01:
            pass
        next(blk02, None)   # deferred blocks: PV bursts into freed slots
        for _ in blk02:
            pass
        transposes(0, 0, get_attnP(0))
        next(blk03, None)
        for _ in blk03:
            pass
        transposes(0, 1, get_attnP(0))

        # ---- phase 2: remaining blocks, software-pipelined 2 deep ----
        blocks = [(q, h) for q in range(NQ) for h in range(4) if q > 0]
        prev_g, prev_qh = None, None
        for bi, (q, h) in enumerate(blocks):
            g = attn_block(q, h, ea_p2, get_attnP(q))
            if prev_g is None:
                for _ in range(4):
                    next(g, None)
            else:
                for _ in range(4):
                    next(prev_g, None)
                    next(g, None)
                for _ in prev_g:
                    pass
                pq, ph = prev_qh
                if ph % 2 == 1:
                    transposes(pq, ph // 2, get_attnP(pq))
            prev_g, prev_qh = g, (q, h)
        for _ in prev_g:
            pass
        transposes(NQ - 1, 1, get_attnP(NQ - 1))
        while pending_tp:
            emit_tp(*pending_tp.pop(0))
        while pending_out:
            q, dc, ceng = pending_out.pop(0)
            outproj_unit(q, dc, "D" if len(pending_out) == 0 else ceng)
        while pending_dma:
            emit_dma(*pending_dma.pop(0))
    nc.compile()
    return nc


def _prep_inputs(x, Wq, bq, Wk, bk, Wv, bv, Wo, bo):
    # straight perm: psum partition p of m-tile m holds W-col
    # (p//32)*64 + 32*m + (p%32)  (head p//32, dk-dim 32m + p%32)
    perm = np.empty(W, dtype=np.int64)
    for m in range(MT):
        p = np.arange(P)
        perm[m * P + p] = (p // 32) * 64 + 32 * m + (p % 32)

    in_maps = []
    for c in range(8):
        b, g = c // 4, c % 4
        cs = slice(g * W, (g + 1) * W)
        xTb = np.ascontiguousarray(x[b].T)
        Wq_l, bq_l = Wq[:, cs][:, perm], bq[cs][perm]
        Wk_l, bk_l = Wk[:, cs][:, perm], bk[cs][perm]
        in_maps.append({
            "xb": xTb.astype(BF16),
            "x8": xTb.astype(E4),
            "wq8": (16.0 * Wq_l).astype(E4),
            "wk8": (16.0 * Wk_l).astype(E4),
            "wv": (16.0 * Wv[:, cs]).astype(BF16),
            "wo": Wo[cs, :].astype(BF16),
            "bq": np.ascontiguousarray(16.0 * bq_l.reshape(MT, P).T),
            "bk": np.ascontiguousarray(16.0 * bk_l.reshape(MT, P).T),
        })
    return in_maps


def kernel(x, Wq, bq, Wk, bk, Wv, bv, Wo, bo):
    x = np.asarray(x, dtype=np.float32)
    Wq, bq = np.asarray(Wq, np.float32), np.asarray(bq, np.float32)
    Wk, bk = np.asarray(Wk, np.float32), np.asarray(bk, np.float32)
    Wv, bv = np.asarray(Wv, np.float32), np.asarray(bv, np.float32)
    Wo, bo = np.asarray(Wo, np.float32), np.asarray(bo, np.float32)

    if "nc" not in _CACHE:
        _CACHE["nc"] = build_nc()
    nc = _CACHE["nc"]

    in_maps = _prep_inputs(x, Wq, bq, Wk, bk, Wv, bv, Wo, bo)
    res = run_bass_kernel_spmd(nc, in_maps, core_ids=list(range(8))).results

    extra = bv @ Wo + bo   # bv folded out of the V projection
    out = np.empty((B, S, D), dtype=np.float32)
    for b in range(B):
        acc = res[4 * b]["out"].astype(np.float32)
        for g in range(1, 4):
            acc += res[4 * b + g]["out"].astype(np.float32)
        out[b] = acc.T + extra
    return out


# revision 48
# speedup vs baseline: 1.2412x; 1.2412x over previous
"""Multi-head attention (B=2, S=2048, D=1024, H=16, Dk=64) on 8 NeuronCores.

Sharding: 2-way data parallel over batch x 4-way tensor parallel over heads.
Core c = 4*b + g handles batch b, head group g (4 heads = 256 cols).
W_o is row-sliced; the 4 partial outputs per batch are summed on host (+bo).

v2: fp8e4 DoubleRow matmuls wherever precision allows (measured rel_err
~1.4e-2 vs the 2e-2 gate):
  - Q/K projections: x and 16*Wq/k in fp8, DR over K=256 ktiles; psum/16+bias
    moved to SBUF-fp8 by ACT Identity (frees DVE).
  - scores: per-head DR with dk=64 split as 2x32 partitions at quadrant
    bases (tile_position rows 0/32/64/96).
  - softmax exp: split across ACT (exact exp -> fp8 out) and DVE/Pool
    (Schraudolph: bits = rint(s*8/ln2 + 55.6) as int8 == e4m3 bits of
    exp(s), one fused tensor_scalar each).
  - P@V: DR with moving pt pairs [128,2,512]; V stored as fp8 A plus
    equal-scale fp8 residual R = fp8(psum - A); both accumulate into one
    psum so V error ~fp8^2. Rowsum via 16.0-const columns in A (zeros in R).
  - V projection stays f32r (error hits output directly); bias via a
    K=1 ones-row matmul. Output projection in bf16.
Output partials are written bf16; host sums the 4 shards + bo.
v2.1: pt pool deepened to 14 bufs (more WAR slack for the prob tiles
feeding P@V).
v2.2: V projection switched from 8 bf16 ktiles to 3-pass fp8 DoubleRow
(xA*wA + xA*wR + xR*wA with host-prepared residuals; reuses the resident
QK x8 tiles as stationary, drops the 4MB bf16 x load for a 2MB fp8
residual). First w8q/x8 loads split in half so the first projection
matmuls start ~2us earlier. 140610 -> 139641 ns on the timeline model,
rel err 1.599e-2.
"""
import numpy as np
import ml_dtypes
from contextlib import ExitStack

import concourse.bass as bass
import concourse.mybir as mybir
import concourse.tile as tile
from concourse import bacc
from concourse.bass_utils import run_bass_kernel_spmd

F32 = mybir.dt.float32
F32R = mybir.dt.float32r
F8 = mybir.dt.float8e4
I8 = mybir.dt.int8
BF = mybir.dt.bfloat16
E4 = ml_dtypes.float8_e4m3
BF16 = ml_dtypes.bfloat16
DR = mybir.MatmulPerfMode.DoubleRow

B, S, D = 2, 2048, 1024
P = 128
W = 256              # local width (4 heads x 64)
MT = 2               # m-tiles of 128
KO8 = D // P         # 8 f32r contraction ktiles
KO4 = D // 256       # 4 DoubleRow ktiles
NQ = S // 512        # 4 sq chunks
NPAIR = S // 256     # 8 sk chunk-pairs
SCALE = 0.125        # 1/sqrt(64)
A_SCH = 8.0 / np.log(2.0) * SCALE   # 1.442695
B_SCH = 55.6

_CACHE = {}


def to_f32r(x: np.ndarray) -> np.ndarray:
    u = np.ascontiguousarray(x, dtype=np.float32).view(np.uint32)
    r = u + np.uint32(0x7FF) + ((u >> np.uint32(12)) & np.uint32(1))
    r &= np.uint32(0xFFFFF000)
    return r.view(np.float32)


# exp engine split: deterministic weighted round-robin (Bresenham).
# Phase-1 interleaved blocks lean DVE/Pool (ACT does the QK moves there);
# phase-2 leans ACT.
class _ExpAssign:
    def __init__(self, wa, wd, wp):
        self.w = {"A": wa, "D": wd, "P": wp}
        self.acc = {"A": 0.0, "D": 0.0, "P": 0.0}

    def next(self):
        for k in self.acc:
            self.acc[k] += self.w[k]
        k = max(self.acc, key=lambda e: self.acc[e])
        self.acc[k] -= sum(self.w.values())
        return k


def build_nc():
    nc = bacc.Bacc("TRN2", target_bir_lowering=False, debug=False, num_devices=8)
    x8 = nc.dram_tensor("x8", [D, S], F8, kind="ExternalInput").ap()
    x8r = nc.dram_tensor("x8r", [D, S], F8, kind="ExternalInput").ap()
    wq8 = nc.dram_tensor("wq8", [D, W], F8, kind="ExternalInput").ap()
    wk8 = nc.dram_tensor("wk8", [D, W], F8, kind="ExternalInput").ap()
    wv8a = nc.dram_tensor("wv8a", [D, W], F8, kind="ExternalInput").ap()
    wv8r = nc.dram_tensor("wv8r", [D, W], F8, kind="ExternalInput").ap()
    wvb = nc.dram_tensor("wvb", [1, W], BF, kind="ExternalInput").ap()
    wo = nc.dram_tensor("wo", [W, D], BF, kind="ExternalInput").ap()
    bq = nc.dram_tensor("bq", [P, MT], F32, kind="ExternalInput").ap()
    bk = nc.dram_tensor("bk", [P, MT], F32, kind="ExternalInput").ap()
    out = nc.dram_tensor("out", [S, D], BF, kind="ExternalOutput").ap()

    x8_r = x8.rearrange("(ko t p) s -> p ko t s", p=P, t=2)
    x8r_r = x8r.rearrange("(ko t p) s -> p ko t s", p=P, t=2)
    wq8_r = wq8.rearrange("(ko t p) w -> p ko t w", p=P, t=2)
    wk8_r = wk8.rearrange("(ko t p) w -> p ko t w", p=P, t=2)
    wv8a_r = wv8a.rearrange("(ko t p) w -> p ko t w", p=P, t=2)
    wv8r_r = wv8r.rearrange("(ko t p) w -> p ko t w", p=P, t=2)
    wo_r = wo.rearrange("(m p) d -> p m d", p=P)

    with tile.TileContext(nc) as tc, ExitStack() as ctx:
        sb = ctx.enter_context(tc.tile_pool(name="sb", bufs=1))
        xp = ctx.enter_context(tc.tile_pool(name="xp", bufs=2))
        ptp = ctx.enter_context(tc.tile_pool(name="ptp", bufs=4))
        cp = ctx.enter_context(tc.tile_pool(name="cp", bufs=8))
        ps = ctx.enter_context(tc.tile_pool(name="ps", bufs=1, space="PSUM"))

        # ---- resident inputs (issue order = DMA priority); first loads
        # split in half so the first QK matmuls start ~2us earlier ----
        w8q_t = sb.tile([P, KO4, 2, W], F8)
        nc.scalar.dma_start(w8q_t[:, 0:2, :, :], wq8_r[:, 0:2, :, :])
        x8_tiles = [xp.tile([P, KO4, 2, 512], F8, tag="x8", name=f"x8_{i}",
                            bufs=4) for i in range(4)]

        def load_x8(qtr):
            sq = slice(qtr * 512, (qtr + 1) * 512)
            nc.sync.dma_start(x8_tiles[qtr][:, 0:2, :, :], x8_r[:, 0:2, :, sq])
            nc.sync.dma_start(x8_tiles[qtr][:, 2:4, :, :], x8_r[:, 2:4, :, sq])

        sq0 = slice(0, 512)
        nc.sync.dma_start(x8_tiles[0][:, 0:2, :, :], x8_r[:, 0:2, :, sq0])
        nc.scalar.dma_start(w8q_t[:, 2:4, :, :], wq8_r[:, 2:4, :, :])
        bq_t = sb.tile([P, MT], F32)
        nc.sync.dma_start(bq_t[:], bq)
        nc.sync.dma_start(x8_tiles[0][:, 2:4, :, :], x8_r[:, 2:4, :, sq0])
        w8k_t = sb.tile([P, KO4, 2, W], F8)
        nc.scalar.dma_start(w8k_t[:, 0:2, :, :], wk8_r[:, 0:2, :, :])
        nc.scalar.dma_start(w8k_t[:, 2:4, :, :], wk8_r[:, 2:4, :, :])
        bk_t = sb.tile([P, MT], F32)
        nc.sync.dma_start(bk_t[:], bk)

        x8r_tiles = [xp.tile([P, KO4, 2, 512], F8, tag="x8r",
                             name=f"x8r_{i}", bufs=4) for i in range(4)]

        def load_xq(qtr):
            sq = slice(qtr * 512, (qtr + 1) * 512)
            nc.sync.dma_start(x8r_tiles[qtr][:], x8r_r[:, :, :, sq])

        load_xq(0)
        wv8a_t = sb.tile([P, KO4, 2, W], F8)
        nc.sync.dma_start(wv8a_t[:], wv8a_r)
        wv8r_t = sb.tile([P, KO4, 2, W], F8)
        nc.sync.dma_start(wv8r_t[:], wv8r_r)
        wvb_t = sb.tile([1, W], BF)
        nc.sync.dma_start(wvb_t[:], wvb)
        wo_t = sb.tile([P, MT, D], BF)
        nc.sync.dma_start(wo_t[:], wo_r)

        ones_t = sb.tile([1, P], BF)
        nc.vector.memset(ones_t[:], 1.0)

        # qt8/kt8: per m-tile [64, 2, S]; head h lives in tile h//2 at
        # partition base 32*(h%2) (base 96 is rejected by base_partition)
        qt8_m = [sb.tile([64, 2, S], F8, name=f"qt8_{m}") for m in range(MT)]
        kt8_m = [sb.tile([64, 2, S], F8, name=f"kt8_{m}") for m in range(MT)]
        # vaug A/R: [sk-part, pair, parity, head*128]; per head 64 V-dims
        # then 64 const cols (A: 16.0 -> rowsum*16; R: 0.0)
        vaugA_t = sb.tile([P, NPAIR, 2, 512], F8)
        vaugR_t = sb.tile([P, NPAIR, 2, 512], F8)
        consA = vaugA_t[:].rearrange("p i t (h c) -> p i t h c", c=P)[:, :, :, :, 64:P]
        nc.gpsimd.memset(consA, 16.0)
        consR = vaugR_t[:].rearrange("p i t (h c) -> p i t h c", c=P)[:, :, :, :, 64:P]
        nc.gpsimd.memset(consR, 0.0)
        ot_t = sb.tile([P, MT, S], BF)

        ea_p1 = _ExpAssign(1.0, 1.0, 0.0)      # phase-1 blocks
        ea_p2 = _ExpAssign(7.0, 6.0, 0.0)      # phase-2 mix (ACT/DVE only)
        ea_pA = _ExpAssign(9.0, 4.0, 0.0)      # boundary: ACT-heavy
        ea_pD = _ExpAssign(5.5, 7.5, 0.0)      # compensation: DVE-heavy

        _uid = [0]
        _pairctr = [0]

        pending_out = []

        def attn_block(q, h, ea):
            _uid[0] += 1
            u = _uid[0]
            sq = slice(q * 512, (q + 1) * 512)
            hq = slice(32 * (h % 2), 32 * (h % 2) + 32)
            qt8, kt8 = qt8_m[h // 2], kt8_m[h // 2]
            hs = slice(h * P, (h + 1) * P)
            oP = ps.tile([P, 512], F32, tag="oP", bufs=2, name=f"oP{u}")

            def attn_v(i, pt):
                has_r = (i % 2 == 0)
                nc.tensor.matmul(oP[:], vaugA_t[:, i, :, hs], pt[:],
                                 start=(i == 0),
                                 stop=(i == NPAIR - 1 and not has_r),
                                 perf_mode=DR)
                if has_r:
                    nc.tensor.matmul(oP[:], vaugR_t[:, i, :, hs], pt[:],
                                     start=False, stop=(i == NPAIR - 1),
                                     perf_mode=DR)

            pend_av = []
            for i in range(NPAIR):
                pt = ptp.tile([P, 2, 512], F8, tag="pt", bufs=14,
                              name=f"pt{u}_{i}")
                sp = ps.tile([P, 1024], F32, tag="sp", bufs=3,
                             name=f"sp{u}_{i}")
                for half in (0, 1):
                    ks = slice((2 * i + half) * P, (2 * i + half + 1) * P)
                    nc.tensor.matmul(sp[:, half * 512:(half + 1) * 512],
                                     kt8[hq, :, ks], qt8[hq, :, sq],
                                     start=True, stop=True, perf_mode=DR)
                ptf = pt[:].rearrange("p t n -> p (t n)")
                eng = ea.next()
                if eng == "A":
                    nc.scalar.activation(ptf, sp[:],
                                         mybir.ActivationFunctionType.Exp,
                                         bias=0.0, scale=SCALE)
                else:
                    nc.vector.tensor_scalar(ptf.bitcast(I8), sp[:],
                                            A_SCH, B_SCH,
                                            mybir.AluOpType.mult,
                                            mybir.AluOpType.add)
                if len(pend_av) >= 4:
                    attn_v(*pend_av.pop(0))
                pend_av.append((i, pt))
                _pairctr[0] += 1
                if pending_out and i >= 4 and (i % 2 == 0
                                               or len(pending_out) >= 6):
                    outproj_unit(*pending_out.pop(0))
                yield i
            while pend_av:
                attn_v(*pend_av.pop(0))
            rs = cp.tile([64, 512], F32, tag="rs", bufs=4, name=f"rs{u}")
            nc.vector.reciprocal(rs[:], oP[64:P, :])
            nc.vector.tensor_mul(ot_t[64 * (h % 2):64 * (h % 2) + 64, h // 2, sq],
                                 oP[0:64, :], rs[:])
            yield NPAIR

        def qk_proj(qtr, m, wt, bt, dst):
            sq = slice(qtr * 512, (qtr + 1) * 512)
            ppt = ps.tile([P, 1024], F32, tag="sp", bufs=3)
            pp = ppt[:, 0:512]
            for ko in range(KO4):
                nc.tensor.matmul(pp, wt[:, ko, :, m * P:(m + 1) * P],
                                 x8_tiles[qtr][:, ko, :, :],
                                 start=(ko == 0), stop=(ko == KO4 - 1),
                                 perf_mode=DR)
            # t0 move on ACT, t1 on DVE — parallel, frees pp sooner
            nc.scalar.activation(
                dst[m][0:64, 0, sq], ppt[0:64, 0:512],
                mybir.ActivationFunctionType.Identity,
                bias=bt[0:64, m:m + 1], scale=0.0625)
            nc.vector.tensor_scalar(dst[m][0:64, 1, sq], ppt[64:P, 0:512],
                                    0.0625, bt[64:P, m:m + 1],
                                    mybir.AluOpType.mult,
                                    mybir.AluOpType.add)

        def v_proj(qtr, st):
            gst = qtr * 4 + st
            pair, par = gst // 2, gst % 2
            pvt = ps.tile([P, 1024], F32, tag="sp", bufs=3)
            pv = pvt[:, 0:W]
            for pss in range(3):   # (xA,wA), (xA,wR), (xR,wA)
                xs = x8_tiles if pss < 2 else x8r_tiles
                ws = wv8r_t if pss == 1 else wv8a_t
                for ko in range(KO4):
                    nc.tensor.matmul(pv, xs[qtr][:, ko, :, st * P:(st + 1) * P],
                                     ws[:, ko, :, :],
                                     start=(pss == 0 and ko == 0), stop=False,
                                     perf_mode=DR)
            nc.tensor.matmul(pv, ones_t[:], wvb_t[:], start=False, stop=True)
            Av = vaugA_t[:, pair, par, :].rearrange("p (h c) -> p h c", c=P)[:, :, 0:64]
            Rv = vaugR_t[:, pair, par, :].rearrange("p (h c) -> p h c", c=P)[:, :, 0:64]
            pvv = pv.rearrange("p (h c) -> p h c", c=64)
            nc.scalar.activation(Av, pvv, mybir.ActivationFunctionType.Copy,
                                 bias=0.0, scale=1.0)
            if pair % 2 == 0:
                nc.vector.scalar_tensor_tensor(Rv, pvv, 1.0, Av,
                                               mybir.AluOpType.mult,
                                               mybir.AluOpType.subtract)

        def outproj_unit(mo, n, ceng):
            po = ps.tile([P, 512], F32, tag="oP", bufs=2)
            for m in range(MT):
                nc.tensor.matmul(po[:],
                                 ot_t[:, m, mo * P:(mo + 1) * P],
                                 wo_t[:, m, n * 512:(n + 1) * 512],
                                 start=(m == 0), stop=(m == MT - 1))
            ob = cp.tile([P, 512], BF, tag="ob", bufs=6)
            if ceng == "A":
                nc.scalar.activation(ob[:], po[:],
                                     mybir.ActivationFunctionType.Copy,
                                     bias=0.0, scale=1.0)
            else:
                nc.vector.tensor_copy(ob[:], po[:])
            nc.sync.dma_start(
                out[mo * P:(mo + 1) * P, n * 512:(n + 1) * 512], ob[:])

        def outproj(q):
            for st in range(4):
                for n in range(2):
                    pending_out.append((q * 4 + st, n, "A"))

        # ---- phase 1: projections with 2 interleaved attention blocks ----
        blk00 = attn_block(0, 0, ea_p1)
        blk01 = attn_block(0, 1, ea_p1)
        for qtr in range(NQ):
            if qtr > 0:
                load_x8(qtr)
                load_xq(qtr)
            for m in range(MT):
                qk_proj(qtr, m, w8q_t, bq_t, qt8_m)
                qk_proj(qtr, m, w8k_t, bk_t, kt8_m)
            next(blk00, None)
            v_proj(qtr, 0)
            next(blk01, None)
            v_proj(qtr, 1)
            next(blk00, None)
            v_proj(qtr, 2)
            next(blk01, None)
            v_proj(qtr, 3)
        next(blk00, None)   # epilogues
        next(blk01, None)

        # ---- phase 2: remaining blocks, software-pipelined 2 deep ----
        blocks = [(q, h) for q in range(NQ) for h in range(4)
                  if not (q == 0 and h < 2)]
        prev_g, prev_qh = None, None
        for bi, (q, h) in enumerate(blocks):
            ea_sel = ea_pA if bi < 2 else (ea_pD if bi < 4 else ea_p2)
            g = attn_block(q, h, ea_sel)
            if prev_g is None:
                for _ in range(4):
                    next(g, None)
            else:
                for _ in range(4):
                    next(prev_g, None)
                    next(g, None)
                for _ in prev_g:
                    pass
                if prev_qh[1] == 3:
                    outproj(prev_qh[0])
            prev_g, prev_qh = g, (q, h)
        for _ in prev_g:
            pass
        outproj(NQ - 1)
        _dr = 0
        while pending_out:
            mo, n, ceng = pending_out.pop(0)
            _dr += 1
            outproj_unit(mo, n, "D" if _dr % 2 == 0 else "A")
    nc.compile()
    return nc


def _prep_inputs(x, Wq, bq, Wk, bk, Wv, bv, Wo, bo):
    # column permutation for the DR-score layout: psum partition p of m-tile m
    # holds W-col (2m + (p%64)//32)*64 + (p//64)*32 + (p%32)
    perm = np.empty(W, dtype=np.int64)
    for m in range(MT):
        p = np.arange(P)
        perm[m * P + p] = (2 * m + (p % 64) // 32) * 64 + (p // 64) * 32 + (p % 32)

    in_maps = []
    for c in range(8):
        b, g = c // 4, c % 4
        cs = slice(g * W, (g + 1) * W)
        xTb = np.ascontiguousarray(x[b].T)
        Wq_l, bq_l = Wq[:, cs][:, perm], bq[cs][perm]
        Wk_l, bk_l = Wk[:, cs][:, perm], bk[cs][perm]
        x8a_np = xTb.astype(E4)
        wv16 = 16.0 * Wv[:, cs]
        wv8a_np = wv16.astype(E4)
        in_maps.append({
            "x8": x8a_np,
            "x8r": (xTb - x8a_np.astype(np.float32)).astype(E4),
            "wq8": (16.0 * Wq_l).astype(E4),
            "wk8": (16.0 * Wk_l).astype(E4),
            "wv8a": wv8a_np,
            "wv8r": (wv16 - wv8a_np.astype(np.float32)).astype(E4),
            "wvb": (16.0 * bv[cs].reshape(1, W)).astype(BF16),
            "wo": Wo[cs, :].astype(BF16),
            "bq": np.ascontiguousarray(bq_l.reshape(MT, P).T),
            "bk": np.ascontiguousarray(bk_l.reshape(MT, P).T),
        })
    return in_maps


def kernel(x, Wq, bq, Wk, bk, Wv, bv, Wo, bo):
    x = np.asarray(x, dtype=np.float32)
    Wq, bq = np.asarray(Wq, np.float32), np.asarray(bq, np.float32)
    Wk, bk = np.asarray(Wk, np.float32), np.asarray(bk, np.float32)
    Wv, bv = np.asarray(Wv, np.float32), np.asarray(bv, np.float32)
    Wo, bo = np.asarray(Wo, np.float32), np.asarray(bo, np.float32)

    if "nc" not in _CACHE:
        _CACHE["nc"] = build_nc()
    nc = _CACHE["nc"]

    in_maps = _prep_inputs(x, Wq, bq, Wk, bk, Wv, bv, Wo, bo)
    res = run_bass_kernel_spmd(nc, in_maps, core_ids=list(range(8))).results

    out = np.empty((B, S, D), dtype=np.float32)
    for b in range(B):
        acc = res[4 * b]["out"].astype(np.float32)
        for g in range(1, 4):
            acc += res[4 * b + g]["out"].astype(np.float32)
        out[b] = acc + bo
    return out

